# revision 20
# baseline (speedup 1.0000x reference)
"""Correlation-volume kernel for trn2 (8 NeuronCores, batch-parallel).

out[n, (i,j), h, w] = sum_z imgA[n,z,h,w] * imgB[n,z,h+(j-4),w+(i-4)]
(zero padding outside the image; equivalent to the bilinear reference
to ~1e-5 relative).

Device strategy (per core, one batch element):
  - inputs cast to fp16 on host; B zero-padded to 168x168 on host; A
    re-laid-out block-major on host (walrus: matmul weights AP must have
    a single free dim).
  - 200 stationary blocks of 16x8=128 A-pixels. Each block runs TWO
    TensorE matmuls that share a PSUM bank:
      lo: stationary = pixels hl 0-7  (64), moving = B window rows
          [16bh, 16bh+16) x 16 cols -> psum[0:64, 0:256]
      hi: stationary = pixels hl 8-15 (64), moving = rows
          [16bh+8, 16bh+24) x 16 cols -> psum[64:128, 0:256]
    The row-origin shift of the hi window aligns both halves' used
    bands into the SAME [0,256) column window: pixel p=(hl,wl) has its
    81 displacement dot-products at cols 16*(hl%8)+wl + 16*dy+dx
    (dy,dx in 0..8), max 119+136=255. PSUM is a dense [128x256] tile:
    one full-width copy per block, and staging slots are per-partition
    CONTIGUOUS (512B x GRP slots), so the spill DMAs ship ~10KB
    descriptors at full HBM rate. Spill = 13.1MB vs 19.7MB full gram
    (the kernel is DMA-pool-bound, so bytes and descriptor size are
    everything).
  - DVE/ACT alternate blocks copying PSUM -> SBUF staging (cast fp16);
    Pool (SWDGE) spills each 20-block group; band -> 81-entry gather
    happens on host during the unshard via fancy indexing.

Sync notes: distinct DMAs complete OUT OF ORDER across queues, so every
DMA that gates compute gets its own semaphore (per input strip-pair, per
staging buffer slot). Engine-side increments (matmul, copies) are
in-order per engine, so single counting sems are fine there.
"""

import numpy as np

import concourse.bass as bass
import concourse.mybir as mybir
from concourse.bass_utils import run_bass_kernel_spmd

F16 = mybir.dt.float16
F32 = mybir.dt.float32

Z = 128
H = W = 160
PAD = 4
R = 9                      # displacements per axis
BH, BW = 16, 8             # stationary block (BH*BW == 128)
NBH, NBW = H // BH, W // BW
NB = NBH * NBW             # 200 blocks
MW = BW + 2 * PAD          # 16 moving-window cols
MROW = 16                  # moving-window rows per half-matmul
BAND = MROW * MW           # 256 psum cols per block
HP, WP = H + 2 * PAD, W + 2 * PAD     # 168 x 168 padded B
NPS = 8                    # PSUM banks in rotation
GRP = 20                   # blocks per spill group (= one block-row)
NSTG = 5                   # staging buffers
NG = NB // GRP             # 10 spill groups
STRIP = 32                 # input load strip (rows)
NWARM = 16                 # PE warmup matmuls (HAM un-throttle)

NP_F16 = np.float16

NBS = (HP + STRIP - 1) // STRIP   # 6 B strips (last is 8 rows)
NAS = H // STRIP                  # 5 A strips


def _strips_needed(bh):
    """(jb, ja): last B strip and last A strip block-row bh depends on."""
    jb = (BH * bh + BH + 2 * PAD - 1) // STRIP
    ja = (BH * bh + BH - 1) // STRIP
    return jb, ja


def build_nc():
    nc = bass.Bass()
    a = nc.declare_dram_parameter("a", [Z, H * W], F16, isOutput=False)
    bp = nc.declare_dram_parameter("bp", [Z, HP * WP], F16, isOutput=False)
    g = nc.declare_dram_parameter("g", [Z, NB * BAND], F16, isOutput=True)

    # one sem per strip index j: B_j incs +16, A_j (j<NAS) incs +16.
    s_ld = [nc.alloc_semaphore(f"s_ld{j}") for j in range(NBS)]
    s_sp = [nc.alloc_semaphore(f"s_sp{i}") for i in range(NSTG)]

    with (
        nc.sbuf_tensor([Z, H * W], F16) as a_sb,
        nc.sbuf_tensor([Z, HP * WP], F16) as b_sb,
        nc.sbuf_tensor([Z, GRP * BAND], F16) as stage0,
        nc.sbuf_tensor([Z, GRP * BAND], F16) as stage1,
        nc.sbuf_tensor([Z, GRP * BAND], F16) as stage2,
        nc.sbuf_tensor([Z, GRP * BAND], F16) as stage3,
        nc.sbuf_tensor([Z, GRP * BAND], F16) as stage4,
        nc.psum_tensor([Z, BAND], F32) as ps0,
        nc.psum_tensor([Z, BAND], F32) as ps1,
        nc.psum_tensor([Z, BAND], F32) as ps2,
        nc.psum_tensor([Z, BAND], F32) as ps3,
        nc.psum_tensor([Z, BAND], F32) as ps4,
        nc.psum_tensor([Z, BAND], F32) as ps5,
        nc.psum_tensor([Z, BAND], F32) as ps6,
        nc.psum_tensor([Z, BAND], F32) as ps7,
        nc.semaphore("s_mm") as s_mm,
        nc.semaphore("s_cpv") as s_cpv,
        nc.semaphore("s_cpa") as s_cpa,
        nc.Block() as block,
    ):
        psum = [ps0, ps1, ps2, ps3, ps4, ps5, ps6, ps7]
        stage = [stage0, stage1, stage2, stage3, stage4]
        b3 = b_sb[:].rearrange("p (h w) -> p h w", h=HP)
        b3d = bp[:].rearrange("p (h w) -> p h w", h=HP)

        @block.sync
        def _(sync):
            # input strip loads (no waits -> issue immediately, FIFO).
            # "a" is block-major on host: a strip of STRIP image rows is
            # a whole number of block rows = contiguous columns.
            for j in range(NBS):
                r0, r1 = j * STRIP, min((j + 1) * STRIP, HP)
                sync.dma_start(
                    out=b3[:, r0:r1, :], in_=b3d[:, r0:r1, :]
                ).then_inc(s_ld[j], 16)
                if j < NAS:
                    c0, c1 = j * STRIP * W, (j + 1) * STRIP * W
                    sync.dma_start(
                        out=a_sb[:, c0:c1], in_=a[:, c0:c1]
                    ).then_inc(s_ld[j], 16)
            for i in range(NSTG):
                nsp = (NG - i + NSTG - 1) // NSTG
                sync.wait_ge(s_sp[i], 16 * nsp)

        @block.gpsimd
        def _(gpsimd):
            # gram spills (group gi -> staging buffer gi%NSTG): one DMA
            # per 20-block group; staging is per-partition contiguous
            # (GRP*512B runs) so descriptors are large (10KB, full HBM
            # rate). Issued from Pool (SWDGE) so the spill stream
            # overlaps the input loads on the sync engine's queue, and
            # the copy engines never stall on spill waits.
            for gi in range(NG):
                nblk = gi * GRP + GRP
                gpsimd.wait_ge(s_cpv, (nblk + 1) // 2)  # even blocks (DVE)
                gpsimd.wait_ge(s_cpa, nblk // 2)        # odd blocks (ACT)
                gpsimd.dma_start(
                    out=g[:, gi * GRP * BAND:(gi + 1) * GRP * BAND],
                    in_=stage[gi % NSTG][:, :],
                ).then_inc(s_sp[gi % NSTG], 16)

        @block.tensor
        def _(tensor):
            # HAM warmup: dense dummy matmuls on scratch data so the PE
            # clock is at 8/8 before the real stream begins. Results land
            # in bank 0, overwritten by block 0 (start=True).
            for _ in range(NWARM):
                nc.tensor.matmul(
                    psum[0][:, :],
                    stage[0][:, 0:128],
                    stage[1][:, 0:BAND],
                    start=True,
                    stop=True,
                )
            waited = set()
            for b in range(NB):
                bh, bw = divmod(b, NBW)
                if bw == 0:
                    jb, ja = _strips_needed(bh)
                    for j in range(jb + 1):
                        if j not in waited:
                            need = 32 if j < NAS else 16
                            tensor.wait_ge(s_ld[j], need)
                            waited.add(j)
                if b >= NPS:
                    pb = b - NPS
                    if pb % 2 == 0:
                        tensor.wait_ge(s_cpv, pb // 2 + 1)
                    else:
                        tensor.wait_ge(s_cpa, pb // 2 + 1)
                h0, w0 = bh * BH, bw * BW
                nc.tensor.matmul(
                    psum[b % NPS][0:64, :],
                    a_sb[:, b * 128:b * 128 + 64],
                    b3[:, h0:h0 + MROW, w0:w0 + MW],
                    start=True,
                    stop=True,
                ).then_inc(s_mm, 1)
                nc.tensor.matmul(
                    psum[b % NPS][64:128, :],
                    a_sb[:, b * 128 + 64:(b + 1) * 128],
                    b3[:, h0 + 8:h0 + 8 + MROW, w0:w0 + MW],
                    start=True,
                    stop=True,
                ).then_inc(s_mm, 1)

        @block.vector
        def _(vector):
            for b in range(0, NB, 2):
                gi, sl = b // GRP, b % GRP
                if sl <= 1 and gi >= NSTG:
                    # staging slot free once its previous spill landed.
                    vector.wait_ge(s_sp[gi % NSTG], 16 * (gi // NSTG))
                vector.wait_ge(s_mm, 2 * (b + 1))
                nc.vector.tensor_copy(
                    stage[gi % NSTG][:, sl * BAND:(sl + 1) * BAND],
                    psum[b % NPS][:, :],
                ).then_inc(s_cpv, 1)

        @block.scalar
        def _(scalar):
            for b in range(1, NB, 2):
                gi, sl = b // GRP, b % GRP
                if sl <= 1 and gi >= NSTG:
                    scalar.wait_ge(s_sp[gi % NSTG], 16 * (gi // NSTG))
                scalar.wait_ge(s_mm, 2 * (b + 1))
                nc.scalar.copy(
                    stage[gi % NSTG][:, sl * BAND:(sl + 1) * BAND],
                    psum[b % NPS][:, :],
                ).then_inc(s_cpa, 1)

    return nc


def prep_core(An, Bn):
    """An, Bn: [Z,H,W] float32 -> per-core input map (fp16, B padded).

    "a" is laid out block-major: [z, bh, bw, h_l, w_l] so each stationary
    block's 128 pixels are contiguous (walrus: weights AP must be 1-D free).
    """
    a = (
        An.reshape(Z, NBH, BH, NBW, BW)
        .transpose(0, 1, 3, 2, 4)
        .reshape(Z, H * W)
        .astype(NP_F16)
    )
    bpad = np.zeros((Z, HP, WP), NP_F16)
    bpad[:, PAD:PAD + H, PAD:PAD + W] = Bn
    return {"a": np.ascontiguousarray(a), "bp": bpad.reshape(Z, HP * WP)}


def extract_core(gres):
    """gres: [Z, NB*BAND] fp16 banded gram spill -> [81,H,W] float32.

    Partition p=(hl,wl) of block (bh,bw) holds, at band col
    16*(hl%8) + wl + 16*(dy+4) + (dx+4), the dot product for pixel
    (16*bh+hl, 8*bw+wl) at displacement (dy, dx).
    """
    G = np.ascontiguousarray(gres).reshape(Z, NBH, NBW, BAND)
    p = np.arange(Z)
    hl, wl = p // BW, p % BW
    base = 16 * (hl % 8) + wl
    out = np.empty((R * R, H, W), np.float32)
    for dx in range(-PAD, PAD + 1):
        for dy in range(-PAD, PAD + 1):
            k = (dx + PAD) * R + (dy + PAD)
            cols = base + 16 * (dy + PAD) + (dx + PAD)
            V = G[p, :, :, cols]                    # [128, NBH, NBW]
            out[k] = (
                V.reshape(BH, BW, NBH, NBW)
                .transpose(2, 0, 3, 1)
                .astype(np.float32)
                .reshape(H, W)
            )
    return out


_NC_CACHE = {}


def get_nc():
    if "nc" not in _NC_CACHE:
        _NC_CACHE["nc"] = build_nc()
    return _NC_CACHE["nc"]


def kernel(imgA, imgB):
    imgA = np.asarray(imgA)
    imgB = np.asarray(imgB)
    N = imgA.shape[0]
    in_maps = [prep_core(imgA[n], imgB[n]) for n in range(N)]
    res = run_bass_kernel_spmd(get_nc(), in_maps, list(range(N)))
    return np.stack([extract_core(res.results[n]["g"]) for n in range(N)])
